# revision 39
# baseline (speedup 1.0000x reference)
"""Trainium2 Bass/Tile kernel for nn_Capsule3D (capsule conv + routing softmax + squash).

Sharding: data-parallel over batch, 2 samples per core x 8 cores. Host side does
only layout transforms (transpose / 9-shift im2col row replication / dtype casts)
and sharding; all math runs on the NeuronCores.

Per sample b, on device (layout: partitions = (c,l) = 128 output channels,
free = output positions pos = 900, per input capsule i = 0..31):
  - t = conv(sum_i x_i) via a mini 72x128 matmul (conv is linear in its input,
    so the routing sum over capsules commutes with the conv)
  - main loop per i: K=72 weights-stationary conv matmul -> PSUM; evict to bf16
    (split ScalarE/VectorE); q = u_hat*t (VectorE bf16 2x); "Lrep" matmul with a
    block-diagonal ones matrix reduces over l AND replicates the result over the
    l partitions; exp((z)/sqrt L) on ScalarE straight from PSUM with accum_out
    giving the softmax denominator column-sums for free; q2 = u_hat*e (VectorE).
  - softmax denominators via gpsimd partition_all_reduce; 1/S_i folded into
    per-i scaled-identity matmuls ("si") built on GpSimd.
  - s = sum_i si^T @ q2_i accumulated in PSUM by TensorE (+ t*b_route term),
    so the s-phase is matmul-only and overlaps the next sample's main loop.
  - squash: norm over l via Lrep matmul on v^2, then v*(1-exp(-r))/r.

The softmax skips the max-subtraction (logits are O(5), safe in fp32 exp).
Intermediates are bf16 (measured end-to-end error ~9e-3 scale-relative absmax
vs the fp32 reference, i.e. ~1e-2 l2-relative, under the 2e-2 gate).
"""

import math

import numpy as np

# ---------------- problem constants (hardcoded per harness contract) ----------
B, H, W, IC, IL = 16, 32, 32, 32, 8
KH = KW = 3
CL = 128
L = 8
C = CL // L            # 16
OH = OW = 30
POS = OH * OW          # 900
HW = H * W             # 1024
K9 = KH * KW * IL      # 72
NCORES = 8
BLOC = B // NCORES     # 2
EPS = 1e-7
RSQRT_L = 1.0 / math.sqrt(float(L))
SHIFTS = [32 * ky + kx for ky in range(KH) for kx in range(KW)]

_CACHE = {}


def _build_nc():
    import concourse.tile as tile
    from concourse import bacc, mybir

    f32 = mybir.dt.float32
    bf16 = mybir.dt.bfloat16
    AF = mybir.ActivationFunctionType
    OP = mybir.AluOpType

    nc = bacc.Bacc()

    xt9_d = nc.dram_tensor("xt9", [BLOC, IC, K9, HW], bf16, kind="ExternalInput")
    xnat_d = nc.dram_tensor("xnat", [BLOC, HW, IC * IL], f32, kind="ExternalInput")
    w72_d = nc.dram_tensor("w72", [K9, CL], bf16, kind="ExternalInput")
    lrep_d = nc.dram_tensor("lrep", [128, 128], bf16, kind="ExternalInput")
    i128_d = nc.dram_tensor("i128", [128, 128], bf16, kind="ExternalInput")
    br_d = nc.dram_tensor("br_cl", [128, POS], f32, kind="ExternalInput")
    y_d = nc.dram_tensor("y", [BLOC, 128, POS], f32, kind="ExternalOutput")

    HP = 450  # half of the 900 output positions

    with tile.TileContext(nc) as tc:
        with (
            tc.tile_pool(name="const", bufs=1) as constp,
            tc.tile_pool(name="xnat", bufs=2) as xnatp,
            tc.tile_pool(name="ub", bufs=4) as ubp,
            tc.tile_pool(name="ubar", bufs=1) as ubarp,
            tc.tile_pool(name="xt9", bufs=4) as xt9p,
            tc.tile_pool(name="utmp", bufs=5) as utmpp,
            tc.tile_pool(name="etmp", bufs=5) as etmpp,
            tc.tile_pool(name="q2s", bufs=IC) as q2p,
            tc.tile_pool(name="tt", bufs=2) as ttp,
            tc.tile_pool(name="q", bufs=8) as qp,
            tc.tile_pool(name="sip", bufs=IC) as sip,
            tc.tile_pool(name="sm", bufs=2) as smp,
            tc.tile_pool(name="sq", bufs=3) as sqp,
            tc.tile_pool(name="pu", bufs=3, space="PSUM") as pup,
            tc.tile_pool(name="pz", bufs=2, space="PSUM") as pzp,
            tc.tile_pool(name="lt", bufs=1, space="PSUM") as ltp,
        ):
            # ---- constants (loaded once) ----
            w72s = constp.tile([K9, CL], bf16)
            nc.sync.dma_start(out=w72s, in_=w72_d[:, :])
            lreps = constp.tile([128, 128], bf16)
            nc.sync.dma_start(out=lreps, in_=lrep_d[:, :])
            i128s = constp.tile([128, 128], bf16)
            nc.sync.dma_start(out=i128s, in_=i128_d[:, :])
            brs = constp.tile([128, POS], f32)
            nc.gpsimd.dma_start(out=brs, in_=br_d[:, :])
            eps_t = constp.tile([128, 1], f32)
            nc.vector.memset(eps_t, EPS)

            for b in range(BLOC):
                # ---------- ubar path ----------
                ubarT = ubarp.tile([IL, HW], bf16, tag="ubarT")
                xn = xnatp.tile([128, HW // 128, IC * IL], f32, tag="xn")
                nc.sync.dma_start(
                    out=xn, in_=xnat_d[b].rearrange("(t p) f -> p t f", p=128)
                )
                for hwt in range(HW // 128):
                    ub_f = ubp.tile([128, IL], f32, tag="ubf")
                    nc.vector.reduce_sum(
                        out=ub_f,
                        in_=xn[:, hwt, :].rearrange("p (i l) -> p l i", l=IL),
                        axis=mybir.AxisListType.X,
                    )
                    ub_b = ubp.tile([128, IL], bf16, tag="ubb")
                    nc.scalar.copy(out=ub_b, in_=ub_f)
                    ps_tr = pup.tile([IL, 128], bf16, tag="pu")
                    nc.tensor.transpose(ps_tr, ub_b, i128s)
                    nc.vector.tensor_copy(
                        out=ubarT[:, hwt * 128 : (hwt + 1) * 128], in_=ps_tr
                    )
                ubar9 = ubarp.tile([K9, HW], bf16, tag="ubar9")
                for g, s in enumerate(SHIFTS):
                    nc.sync.dma_start(
                        out=ubar9[g * IL : (g + 1) * IL, 0 : HW - s],
                        in_=ubarT[:, s:HW],
                    )

                # ---------- t mini-conv (per half) ----------
                ub_v = ubar9.rearrange("p (h w) -> p h w", w=W)
                t_bf = ttp.tile([128, 2, HP], bf16, tag="tbf")
                t2_f = ttp.tile([128, 2, HP], f32, tag="t2")
                brv = brs.rearrange("p (h n) -> p h n", h=2)
                for h in range(2):
                    psum_t = pup.tile([128, 512], f32, tag="pu")
                    nc.tensor.matmul(
                        psum_t[:, 0:HP],
                        w72s,
                        ub_v[:, 15 * h : 15 * h + 15, 0:OW],
                        start=True,
                        stop=True,
                    )
                    nc.scalar.copy(out=t_bf[:, h, :], in_=psum_t[:, 0:HP])
                    nc.vector.tensor_mul(
                        out=t2_f[:, h, :], in0=psum_t[:, 0:HP], in1=brv[:, h, :]
                    )

                # ---------- main loop: conv, evict, z, l-reduce, exp, q2 ----------
                q2_tiles = []
                colsum = smp.tile([128, IC], f32, tag="colsum")
                for i in range(IC):
                    xt9 = xt9p.tile([K9, HW], bf16, tag="xt9")
                    nc.sync.dma_start(out=xt9, in_=xt9_d[b, i])
                    xv = xt9.rearrange("p (h w) -> p h w", w=W)
                    U_i = utmpp.tile([128, 2, HP], bf16, tag="ut")
                    e_i = etmpp.tile([128, 2, HP], bf16, tag="et")
                    pz = pzp.tile([128, 2, 512], f32, tag="pz")
                    for h in range(2):
                        pu = pup.tile([128, 512], f32, tag="pu")
                        nc.tensor.matmul(
                            pu[:, 0:HP],
                            w72s,
                            xv[:, 15 * h : 15 * h + 15, 0:OW],
                            start=True,
                            stop=True,
                        )
                        if (2 * i + h) % 2 == 0:
                            nc.vector.tensor_copy(out=U_i[:, h, :], in_=pu[:, 0:HP])
                        else:
                            nc.scalar.copy(out=U_i[:, h, :], in_=pu[:, 0:HP])
                    q = qp.tile([128, 2, HP], bf16, tag="q")
                    nc.vector.tensor_mul(out=q, in0=U_i, in1=t_bf)
                    for h in range(2):
                        nc.tensor.matmul(
                            pz[:, h, 0:HP], lreps, q[:, h, :], start=True, stop=True
                        )
                    nc.scalar.activation(
                        out=e_i,
                        in_=pz[:, :, 0:HP],
                        func=AF.Exp,
                        scale=RSQRT_L,
                        accum_out=colsum[:, i : i + 1],
                    )
                    q2_i = q2p.tile([128, 2, HP], bf16, tag="q2", name=f"q2_{i}")
                    q2_tiles.append(q2_i)
                    nc.vector.tensor_mul(out=q2_i, in0=U_i, in1=e_i)

                # ---------- softmax denominator: allreduce over partitions ----------
                from concourse import bass_isa

                s_all = smp.tile([128, IC], f32, tag="sall")
                nc.gpsimd.partition_all_reduce(
                    s_all, colsum, 128, bass_isa.ReduceOp.add
                )
                sinv_tab = smp.tile([128, IC], f32, tag="stab")
                nc.vector.reciprocal(out=sinv_tab, in_=s_all)

                # ---------- s phase + squash, one half at a time ----------
                si_tiles = []
                si_eng = nc.gpsimd if b == 0 else nc.vector
                for i in range(IC):
                    si = sip.tile([128, 128], bf16, tag="si", name=f"si{i}")
                    si_eng.tensor_scalar(
                        out=si,
                        in0=i128s,
                        scalar1=sinv_tab[:, i : i + 1],
                        scalar2=float(L),
                        op0=OP.mult,
                        op1=OP.mult,
                    )
                    si_tiles.append(si)
                o_t = sqp.tile([128, 2, HP], f32, tag="ot")
                for h in range(2):
                    psum_s = ltp.tile([128, 512], f32, tag="lt")
                    for i in range(IC):
                        nc.tensor.matmul(
                            psum_s[:, 0:HP],
                            si_tiles[i],
                            q2_tiles[i][:, h, :],
                            start=(i == 0),
                            stop=(i == IC - 1),
                        )
                    v_sb = sqp.tile([128, HP], f32, tag="vsb")
                    nc.vector.tensor_add(
                        out=v_sb, in0=psum_s[:, 0:HP], in1=t2_f[:, h, :]
                    )
                    sq_bf = sqp.tile([128, HP], bf16, tag="sqbf")
                    nc.scalar.activation(out=sq_bf, in_=v_sb, func=AF.Square)
                    pn = ltp.tile([128, 512], f32, tag="lt")
                    nc.tensor.matmul(
                        pn[:, 0:HP], lreps, sq_bf, start=True, stop=True
                    )
                    rsb = sqp.tile([128, HP], f32, tag="rsb")
                    nc.scalar.activation(
                        out=rsb, in_=pn[:, 0:HP], func=AF.Sqrt, bias=eps_t
                    )
                    rinv = sqp.tile([128, HP], f32, tag="rinv")
                    nc.vector.reciprocal(out=rinv, in_=rsb)
                    g_t = sqp.tile([128, HP], f32, tag="gt")
                    nc.scalar.activation(out=g_t, in_=rsb, func=AF.Exp, scale=-1.0)
                    nc.vector.tensor_scalar(
                        out=g_t,
                        in0=g_t,
                        scalar1=-1.0,
                        scalar2=1.0,
                        op0=OP.mult,
                        op1=OP.add,
                    )
                    a_t = sqp.tile([128, HP], f32, tag="at")
                    nc.vector.tensor_mul(out=a_t, in0=v_sb, in1=rinv)
                    nc.vector.tensor_mul(out=o_t[:, h, :], in0=a_t, in1=g_t)
                nc.gpsimd.dma_start(
                    out=y_d[b].rearrange("p (h n) -> p h n", h=2), in_=o_t
                )

    nc.finalize()
    return nc


def _prep_host(x, w, b_route):
    import ml_dtypes

    bf = ml_dtypes.bfloat16
    x = np.ascontiguousarray(np.asarray(x, dtype=np.float32))
    w = np.asarray(w, dtype=np.float32)
    b_route = np.asarray(b_route, dtype=np.float32)

    # xt[b, i, il, hw]
    xt = np.ascontiguousarray(x.transpose(0, 3, 4, 1, 2)).reshape(B, IC, IL, HW)
    xt9 = np.zeros((B, IC, K9, HW), dtype=bf)
    xtb = xt.astype(bf)
    for g, s in enumerate(SHIFTS):
        if s == 0:
            xt9[:, :, g * IL : (g + 1) * IL, :] = xtb
        else:
            xt9[:, :, g * IL : (g + 1) * IL, : HW - s] = xtb[:, :, :, s:]

    xnat = x.reshape(B, HW, IC * IL)

    # W72[(ky,kx,il), cl]
    w72 = np.ascontiguousarray(
        w[:, :, :, 0, :].transpose(1, 2, 0, 3).reshape(K9, CL)
    ).astype(bf)
    lrep = np.kron(np.eye(C, dtype=np.float32), np.ones((L, L), np.float32)).astype(bf)
    i128 = np.eye(128, dtype=np.float32).astype(bf)
    # br_cl[(c*8+l), pos] = b_route[pos*16+c, l]
    br_cl = np.ascontiguousarray(
        b_route.reshape(POS, C, L).transpose(1, 2, 0).reshape(128, POS)
    ).astype(np.float32)
    return xt9, xnat, w72, lrep, i128, br_cl


def kernel(x, w, b_route, stride):
    assert int(stride) == 1
    xt9, xnat, w72, lrep, i128, br_cl = _prep_host(x, w, b_route)

    if "nc" not in _CACHE:
        _CACHE["nc"] = _build_nc()
    nc = _CACHE["nc"]

    from concourse.bass_utils import run_bass_kernel_spmd

    in_maps = []
    for c in range(NCORES):
        sl = slice(c * BLOC, (c + 1) * BLOC)
        in_maps.append(
            {
                "xt9": np.ascontiguousarray(xt9[sl]),
                "xnat": np.ascontiguousarray(xnat[sl]),
                "w72": w72,
                "lrep": lrep,
                "i128": i128,
                "br_cl": br_cl,
            }
        )

    res = run_bass_kernel_spmd(nc, in_maps, core_ids=list(range(NCORES)))

    y = np.empty((B, OH, OW, C, L), dtype=np.float32)
    for c in range(NCORES):
        yd = res.results[c]["y"]  # [BLOC, 128, 900]
        y[c * BLOC : (c + 1) * BLOC] = (
            yd.reshape(BLOC, C, L, POS).transpose(0, 3, 1, 2).reshape(
                BLOC, OH, OW, C, L
            )
        )
    return y


# revision 40
# speedup vs baseline: 1.0063x; 1.0063x over previous
"""Trainium2 Bass/Tile kernel for nn_Capsule3D (capsule conv + routing softmax + squash).

Sharding: data-parallel over batch, 2 samples per core x 8 cores. Host side does
only layout transforms (transpose / 9-shift im2col row replication / dtype casts)
and sharding; all math runs on the NeuronCores.

Per sample b, on device (layout: partitions = (c,l) = 128 output channels,
free = output positions pos = 900, per input capsule i = 0..31):
  - t = conv(sum_i x_i) via a mini 72x128 matmul (conv is linear in its input,
    so the routing sum over capsules commutes with the conv)
  - main loop per i: K=72 weights-stationary conv matmul -> PSUM; evict to bf16
    (split ScalarE/VectorE); q = u_hat*t (VectorE bf16 2x); "Lrep" matmul with a
    block-diagonal ones matrix reduces over l AND replicates the result over the
    l partitions; exp((z)/sqrt L) on ScalarE straight from PSUM with accum_out
    giving the softmax denominator column-sums for free; q2 = u_hat*e (VectorE).
  - softmax denominators via gpsimd partition_all_reduce; 1/S_i folded into
    per-i scaled-identity matmuls ("si") built on GpSimd.
  - s = sum_i si^T @ q2_i accumulated in PSUM by TensorE (+ t*b_route term),
    so the s-phase is matmul-only and overlaps the next sample's main loop.
  - squash: norm over l via Lrep matmul on v^2, then v*(1-exp(-r))/r.

The softmax skips the max-subtraction (logits are O(5), safe in fp32 exp).
Intermediates are bf16 (measured end-to-end error ~9e-3 scale-relative absmax
vs the fp32 reference, i.e. ~1e-2 l2-relative, under the 2e-2 gate).
"""

import math

import numpy as np

# ---------------- problem constants (hardcoded per harness contract) ----------
B, H, W, IC, IL = 16, 32, 32, 32, 8
KH = KW = 3
CL = 128
L = 8
C = CL // L            # 16
OH = OW = 30
POS = OH * OW          # 900
HW = H * W             # 1024
K9 = KH * KW * IL      # 72
NCORES = 8
BLOC = B // NCORES     # 2
EPS = 1e-7
RSQRT_L = 1.0 / math.sqrt(float(L))
SHIFTS = [32 * ky + kx for ky in range(KH) for kx in range(KW)]

_CACHE = {}


def _build_nc():
    import concourse.tile as tile
    from concourse import bacc, mybir

    f32 = mybir.dt.float32
    bf16 = mybir.dt.bfloat16
    AF = mybir.ActivationFunctionType
    OP = mybir.AluOpType

    nc = bacc.Bacc()

    xt9_d = nc.dram_tensor("xt9", [BLOC, IC, K9, HW], bf16, kind="ExternalInput")
    xnat_d = nc.dram_tensor("xnat", [BLOC, HW, IC * IL], f32, kind="ExternalInput")
    w72_d = nc.dram_tensor("w72", [K9, CL], bf16, kind="ExternalInput")
    lrep_d = nc.dram_tensor("lrep", [128, 128], bf16, kind="ExternalInput")
    i128_d = nc.dram_tensor("i128", [128, 128], bf16, kind="ExternalInput")
    br_d = nc.dram_tensor("br_cl", [128, POS], f32, kind="ExternalInput")
    y_d = nc.dram_tensor("y", [BLOC, 128, POS], f32, kind="ExternalOutput")

    HP = 450  # half of the 900 output positions

    with tile.TileContext(nc) as tc:
        with (
            tc.tile_pool(name="const", bufs=1) as constp,
            tc.tile_pool(name="xnat", bufs=2) as xnatp,
            tc.tile_pool(name="ub", bufs=4) as ubp,
            tc.tile_pool(name="ubar", bufs=1) as ubarp,
            tc.tile_pool(name="xt9", bufs=4) as xt9p,
            tc.tile_pool(name="utmp", bufs=5) as utmpp,
            tc.tile_pool(name="etmp", bufs=5) as etmpp,
            tc.tile_pool(name="q2s", bufs=IC) as q2p,
            tc.tile_pool(name="tt", bufs=2) as ttp,
            tc.tile_pool(name="q", bufs=8) as qp,
            tc.tile_pool(name="sip", bufs=IC) as sip,
            tc.tile_pool(name="sm", bufs=2) as smp,
            tc.tile_pool(name="sq", bufs=3) as sqp,
            tc.tile_pool(name="pu", bufs=3, space="PSUM") as pup,
            tc.tile_pool(name="pz", bufs=2, space="PSUM") as pzp,
            tc.tile_pool(name="lt", bufs=1, space="PSUM") as ltp,
        ):
            # ---- constants (loaded once) ----
            w72s = constp.tile([K9, CL], bf16)
            nc.sync.dma_start(out=w72s, in_=w72_d[:, :])
            lreps = constp.tile([128, 128], bf16)
            nc.sync.dma_start(out=lreps, in_=lrep_d[:, :])
            i128s = constp.tile([128, 128], bf16)
            nc.sync.dma_start(out=i128s, in_=i128_d[:, :])
            brs = constp.tile([128, POS], f32)
            nc.gpsimd.dma_start(out=brs, in_=br_d[:, :])
            eps_t = constp.tile([128, 1], f32)
            nc.vector.memset(eps_t, EPS)
            eye_f = constp.tile([128, 128], f32)
            nc.vector.tensor_copy(out=eye_f, in_=i128s)

            for b in range(BLOC):
                # ---------- ubar path ----------
                ubarT = ubarp.tile([IL, HW], bf16, tag="ubarT")
                xn = xnatp.tile([128, HW // 128, IC * IL], f32, tag="xn")
                nc.sync.dma_start(
                    out=xn, in_=xnat_d[b].rearrange("(t p) f -> p t f", p=128)
                )
                for hwt in range(HW // 128):
                    ub_f = ubp.tile([128, IL], f32, tag="ubf")
                    nc.vector.reduce_sum(
                        out=ub_f,
                        in_=xn[:, hwt, :].rearrange("p (i l) -> p l i", l=IL),
                        axis=mybir.AxisListType.X,
                    )
                    ps_tr = pup.tile([IL, 128], f32, tag="pu")
                    nc.tensor.transpose(ps_tr, ub_f, eye_f)
                    nc.vector.tensor_copy(
                        out=ubarT[:, hwt * 128 : (hwt + 1) * 128], in_=ps_tr
                    )
                ubar9 = ubarp.tile([K9, HW], bf16, tag="ubar9")
                for g, s in enumerate(SHIFTS):
                    nc.sync.dma_start(
                        out=ubar9[g * IL : (g + 1) * IL, 0 : HW - s],
                        in_=ubarT[:, s:HW],
                    )

                # ---------- t mini-conv (per half) ----------
                ub_v = ubar9.rearrange("p (h w) -> p h w", w=W)
                t_bf = ttp.tile([128, 2, HP], bf16, tag="tbf")
                t2_f = ttp.tile([128, 2, HP], f32, tag="t2")
                brv = brs.rearrange("p (h n) -> p h n", h=2)
                psum_t = pzp.tile([128, 2, 512], f32, tag="pz")
                for h in range(2):
                    nc.tensor.matmul(
                        psum_t[:, h, 0:HP],
                        w72s,
                        ub_v[:, 15 * h : 15 * h + 15, 0:OW],
                        start=True,
                        stop=True,
                    )
                    nc.scalar.copy(out=t_bf[:, h, :], in_=psum_t[:, h, 0:HP])
                    nc.vector.tensor_mul(
                        out=t2_f[:, h, :], in0=psum_t[:, h, 0:HP], in1=brv[:, h, :]
                    )

                # ---------- main loop: conv, evict, z, l-reduce, exp, q2 ----------
                q2_tiles = []
                colsum = smp.tile([128, IC], f32, tag="colsum")
                for i in range(IC):
                    xt9 = xt9p.tile([K9, HW], bf16, tag="xt9")
                    nc.sync.dma_start(out=xt9, in_=xt9_d[b, i])
                    xv = xt9.rearrange("p (h w) -> p h w", w=W)
                    U_i = utmpp.tile([128, 2, HP], bf16, tag="ut")
                    e_i = etmpp.tile([128, 2, HP], bf16, tag="et")
                    pz = pzp.tile([128, 2, 512], f32, tag="pz")
                    for h in range(2):
                        pu = pup.tile([128, 512], f32, tag="pu")
                        nc.tensor.matmul(
                            pu[:, 0:HP],
                            w72s,
                            xv[:, 15 * h : 15 * h + 15, 0:OW],
                            start=True,
                            stop=True,
                        )
                        if (2 * i + h) % 2 == 0:
                            nc.vector.tensor_copy(out=U_i[:, h, :], in_=pu[:, 0:HP])
                        else:
                            nc.scalar.copy(out=U_i[:, h, :], in_=pu[:, 0:HP])
                    q = qp.tile([128, 2, HP], bf16, tag="q")
                    nc.vector.tensor_mul(out=q, in0=U_i, in1=t_bf)
                    for h in range(2):
                        nc.tensor.matmul(
                            pz[:, h, 0:HP], lreps, q[:, h, :], start=True, stop=True
                        )
                    nc.scalar.activation(
                        out=e_i,
                        in_=pz[:, :, 0:HP],
                        func=AF.Exp,
                        scale=RSQRT_L,
                        accum_out=colsum[:, i : i + 1],
                    )
                    q2_i = q2p.tile([128, 2, HP], bf16, tag="q2", name=f"q2_{i}")
                    q2_tiles.append(q2_i)
                    nc.vector.tensor_mul(out=q2_i, in0=U_i, in1=e_i)

                # ---------- softmax denominator: allreduce over partitions ----------
                from concourse import bass_isa

                s_all = smp.tile([128, IC], f32, tag="sall")
                nc.gpsimd.partition_all_reduce(
                    s_all, colsum, 128, bass_isa.ReduceOp.add
                )
                sinv_tab = smp.tile([128, IC], f32, tag="stab")
                nc.vector.reciprocal(out=sinv_tab, in_=s_all)

                # ---------- s phase + squash, one half at a time ----------
                si_tiles = []
                si_eng = nc.gpsimd if b == 0 else nc.vector
                for i in range(IC):
                    si = sip.tile([128, 128], bf16, tag="si", name=f"si{i}")
                    si_eng.tensor_scalar(
                        out=si,
                        in0=i128s,
                        scalar1=sinv_tab[:, i : i + 1],
                        scalar2=float(L),
                        op0=OP.mult,
                        op1=OP.mult,
                    )
                    si_tiles.append(si)
                o_t = sqp.tile([128, 2, HP], f32, tag="ot")
                for h in range(2):
                    psum_s = ltp.tile([128, 512], f32, tag="lt")
                    for i in range(IC):
                        nc.tensor.matmul(
                            psum_s[:, 0:HP],
                            si_tiles[i],
                            q2_tiles[i][:, h, :],
                            start=(i == 0),
                            stop=(i == IC - 1),
                        )
                    v_sb = sqp.tile([128, HP], f32, tag="vsb")
                    nc.vector.tensor_add(
                        out=v_sb, in0=psum_s[:, 0:HP], in1=t2_f[:, h, :]
                    )
                    sq_bf = sqp.tile([128, HP], bf16, tag="sqbf")
                    nc.scalar.activation(out=sq_bf, in_=v_sb, func=AF.Square)
                    pn = ltp.tile([128, 512], f32, tag="lt")
                    nc.tensor.matmul(
                        pn[:, 0:HP], lreps, sq_bf, start=True, stop=True
                    )
                    rsb = sqp.tile([128, HP], f32, tag="rsb")
                    nc.scalar.activation(
                        out=rsb, in_=pn[:, 0:HP], func=AF.Sqrt, bias=eps_t
                    )
                    rinv = sqp.tile([128, HP], f32, tag="rinv")
                    nc.vector.reciprocal(out=rinv, in_=rsb)
                    g_t = sqp.tile([128, HP], f32, tag="gt")
                    nc.scalar.activation(out=g_t, in_=rsb, func=AF.Exp, scale=-1.0)
                    nc.vector.tensor_scalar(
                        out=g_t,
                        in0=g_t,
                        scalar1=-1.0,
                        scalar2=1.0,
                        op0=OP.mult,
                        op1=OP.add,
                    )
                    a_t = sqp.tile([128, HP], f32, tag="at")
                    nc.vector.tensor_mul(out=a_t, in0=v_sb, in1=rinv)
                    nc.vector.tensor_mul(out=o_t[:, h, :], in0=a_t, in1=g_t)
                nc.gpsimd.dma_start(
                    out=y_d[b].rearrange("p (h n) -> p h n", h=2), in_=o_t
                )

    nc.finalize()
    return nc


def _prep_host(x, w, b_route):
    import ml_dtypes

    bf = ml_dtypes.bfloat16
    x = np.ascontiguousarray(np.asarray(x, dtype=np.float32))
    w = np.asarray(w, dtype=np.float32)
    b_route = np.asarray(b_route, dtype=np.float32)

    # xt[b, i, il, hw]
    xt = np.ascontiguousarray(x.transpose(0, 3, 4, 1, 2)).reshape(B, IC, IL, HW)
    xt9 = np.zeros((B, IC, K9, HW), dtype=bf)
    xtb = xt.astype(bf)
    for g, s in enumerate(SHIFTS):
        if s == 0:
            xt9[:, :, g * IL : (g + 1) * IL, :] = xtb
        else:
            xt9[:, :, g * IL : (g + 1) * IL, : HW - s] = xtb[:, :, :, s:]

    xnat = x.reshape(B, HW, IC * IL)

    # W72[(ky,kx,il), cl]
    w72 = np.ascontiguousarray(
        w[:, :, :, 0, :].transpose(1, 2, 0, 3).reshape(K9, CL)
    ).astype(bf)
    lrep = np.kron(np.eye(C, dtype=np.float32), np.ones((L, L), np.float32)).astype(bf)
    i128 = np.eye(128, dtype=np.float32).astype(bf)
    # br_cl[(c*8+l), pos] = b_route[pos*16+c, l]
    br_cl = np.ascontiguousarray(
        b_route.reshape(POS, C, L).transpose(1, 2, 0).reshape(128, POS)
    ).astype(np.float32)
    return xt9, xnat, w72, lrep, i128, br_cl


def kernel(x, w, b_route, stride):
    assert int(stride) == 1
    xt9, xnat, w72, lrep, i128, br_cl = _prep_host(x, w, b_route)

    if "nc" not in _CACHE:
        _CACHE["nc"] = _build_nc()
    nc = _CACHE["nc"]

    from concourse.bass_utils import run_bass_kernel_spmd

    in_maps = []
    for c in range(NCORES):
        sl = slice(c * BLOC, (c + 1) * BLOC)
        in_maps.append(
            {
                "xt9": np.ascontiguousarray(xt9[sl]),
                "xnat": np.ascontiguousarray(xnat[sl]),
                "w72": w72,
                "lrep": lrep,
                "i128": i128,
                "br_cl": br_cl,
            }
        )

    res = run_bass_kernel_spmd(nc, in_maps, core_ids=list(range(NCORES)))

    y = np.empty((B, OH, OW, C, L), dtype=np.float32)
    for c in range(NCORES):
        yd = res.results[c]["y"]  # [BLOC, 128, 900]
        y[c * BLOC : (c + 1) * BLOC] = (
            yd.reshape(BLOC, C, L, POS).transpose(0, 3, 1, 2).reshape(
                BLOC, OH, OW, C, L
            )
        )
    return y


# revision 42
# speedup vs baseline: 1.0163x; 1.0099x over previous
"""Trainium2 Bass/Tile kernel for nn_Capsule3D (capsule conv + routing softmax + squash).

Sharding: data-parallel over batch, 2 samples per core x 8 cores. Host side does
only layout transforms (transpose / 9-shift im2col row replication / dtype casts)
and sharding; all math runs on the NeuronCores.

Per sample b, on device (layout: partitions = (c,l) = 128 output channels,
free = output positions pos = 900, per input capsule i = 0..31):
  - t = conv(sum_i x_i) via a mini 72x128 matmul (conv is linear in its input,
    so the routing sum over capsules commutes with the conv)
  - main loop per i: K=72 weights-stationary conv matmul -> PSUM; evict to bf16
    (split ScalarE/VectorE); q = u_hat*t (VectorE bf16 2x); "Lrep" matmul with a
    block-diagonal ones matrix reduces over l AND replicates the result over the
    l partitions; exp((z)/sqrt L) on ScalarE straight from PSUM with accum_out
    giving the softmax denominator column-sums for free; q2 = u_hat*e (VectorE).
  - softmax denominators via gpsimd partition_all_reduce; 1/S_i folded into
    per-i scaled-identity matmuls ("si") built on GpSimd.
  - s = sum_i si^T @ q2_i accumulated in PSUM by TensorE (+ t*b_route term),
    so the s-phase is matmul-only and overlaps the next sample's main loop.
  - squash: norm over l via Lrep matmul on v^2, then v*(1-exp(-r))/r.

The softmax skips the max-subtraction (logits are O(5), safe in fp32 exp).
Intermediates are bf16 (measured end-to-end error ~9e-3 scale-relative absmax
vs the fp32 reference, i.e. ~1e-2 l2-relative, under the 2e-2 gate).
"""

import math

import numpy as np

# ---------------- problem constants (hardcoded per harness contract) ----------
B, H, W, IC, IL = 16, 32, 32, 32, 8
KH = KW = 3
CL = 128
L = 8
C = CL // L            # 16
OH = OW = 30
POS = OH * OW          # 900
HW = H * W             # 1024
K9 = KH * KW * IL      # 72
NCORES = 8
BLOC = B // NCORES     # 2
EPS = 1e-7
RSQRT_L = 1.0 / math.sqrt(float(L))
SHIFTS = [32 * ky + kx for ky in range(KH) for kx in range(KW)]

_CACHE = {}


def _build_nc():
    import concourse.tile as tile
    from concourse import bacc, mybir

    f32 = mybir.dt.float32
    bf16 = mybir.dt.bfloat16
    AF = mybir.ActivationFunctionType
    OP = mybir.AluOpType

    nc = bacc.Bacc()

    xt9_d = nc.dram_tensor("xt9", [BLOC, IC, K9, HW], bf16, kind="ExternalInput")
    xnat_d = nc.dram_tensor("xnat", [BLOC, HW, IC * IL], f32, kind="ExternalInput")
    w72_d = nc.dram_tensor("w72", [K9, CL], bf16, kind="ExternalInput")
    lrep_d = nc.dram_tensor("lrep", [128, 128], bf16, kind="ExternalInput")
    i128_d = nc.dram_tensor("i128", [128, 128], bf16, kind="ExternalInput")
    br_d = nc.dram_tensor("br_cl", [128, POS], f32, kind="ExternalInput")
    y_d = nc.dram_tensor("y", [BLOC, 128, POS], f32, kind="ExternalOutput")

    HP = 450  # half of the 900 output positions

    with tile.TileContext(nc) as tc:
        with (
            tc.tile_pool(name="const", bufs=1) as constp,
            tc.tile_pool(name="xnat", bufs=2) as xnatp,
            tc.tile_pool(name="ub", bufs=4) as ubp,
            tc.tile_pool(name="ubar", bufs=1) as ubarp,
            tc.tile_pool(name="xt9", bufs=4) as xt9p,
            tc.tile_pool(name="utmp", bufs=5) as utmpp,
            tc.tile_pool(name="etmp", bufs=5) as etmpp,
            tc.tile_pool(name="q2s", bufs=IC) as q2p,
            tc.tile_pool(name="tt", bufs=2) as ttp,
            tc.tile_pool(name="q", bufs=8) as qp,
            tc.tile_pool(name="sip", bufs=IC) as sip,
            tc.tile_pool(name="sm", bufs=2) as smp,
            tc.tile_pool(name="sq", bufs=3) as sqp,
            tc.tile_pool(name="pu", bufs=3, space="PSUM") as pup,
            tc.tile_pool(name="pz", bufs=2, space="PSUM") as pzp,
            tc.tile_pool(name="lt", bufs=1, space="PSUM") as ltp,
        ):
            # ---- constants (loaded once) ----
            w72s = constp.tile([K9, CL], bf16)
            nc.sync.dma_start(out=w72s, in_=w72_d[:, :])
            lreps = constp.tile([128, 128], bf16)
            nc.sync.dma_start(out=lreps, in_=lrep_d[:, :])
            i128s = constp.tile([128, 128], bf16)
            nc.sync.dma_start(out=i128s, in_=i128_d[:, :])
            brs = constp.tile([128, POS], f32)
            nc.gpsimd.dma_start(out=brs, in_=br_d[:, :])
            eps_t = constp.tile([128, 1], f32)
            nc.vector.memset(eps_t, EPS)
            eye_f = constp.tile([128, 128], f32)
            nc.vector.tensor_copy(out=eye_f, in_=i128s)

            for b in range(BLOC):
                # ---------- ubar path ----------
                ubarT = ubarp.tile([IL, HW], bf16, tag="ubarT")
                xn = xnatp.tile([128, HW // 128, IC * IL], f32, tag="xn")
                nc.sync.dma_start(
                    out=xn, in_=xnat_d[b].rearrange("(t p) f -> p t f", p=128)
                )
                for hwt in range(HW // 128):
                    ub_f = ubp.tile([128, IL], f32, tag="ubf")
                    nc.vector.reduce_sum(
                        out=ub_f,
                        in_=xn[:, hwt, :].rearrange("p (i l) -> p l i", l=IL),
                        axis=mybir.AxisListType.X,
                    )
                    ps_tr = pup.tile([IL, 128], f32, tag="pu")
                    nc.tensor.transpose(ps_tr, ub_f, eye_f)
                    if hwt % 2 == 0:
                        nc.vector.tensor_copy(
                            out=ubarT[:, hwt * 128 : (hwt + 1) * 128], in_=ps_tr
                        )
                    else:
                        nc.scalar.copy(
                            out=ubarT[:, hwt * 128 : (hwt + 1) * 128], in_=ps_tr
                        )
                ubar9 = ubarp.tile([K9, HW], bf16, tag="ubar9")
                for g, s in enumerate(SHIFTS):
                    nc.sync.dma_start(
                        out=ubar9[g * IL : (g + 1) * IL, 0 : HW - s],
                        in_=ubarT[:, s:HW],
                    )

                # ---------- t mini-conv (per half) ----------
                ub_v = ubar9.rearrange("p (h w) -> p h w", w=W)
                t_bf = ttp.tile([128, 2, HP], bf16, tag="tbf")
                t2_f = ttp.tile([128, 2, HP], f32, tag="t2")
                brv = brs.rearrange("p (h n) -> p h n", h=2)
                psum_t = pzp.tile([128, 2, 512], f32, tag="pz")
                for h in range(2):
                    nc.tensor.matmul(
                        psum_t[:, h, 0:HP],
                        w72s,
                        ub_v[:, 15 * h : 15 * h + 15, 0:OW],
                        start=True,
                        stop=True,
                    )
                    nc.scalar.copy(out=t_bf[:, h, :], in_=psum_t[:, h, 0:HP])
                    nc.vector.tensor_mul(
                        out=t2_f[:, h, :], in0=psum_t[:, h, 0:HP], in1=brv[:, h, :]
                    )

                # ---------- main loop: conv, evict, z, l-reduce, exp, q2 ----------
                q2_tiles = []
                colsum = smp.tile([128, IC], f32, tag="colsum")
                for i in range(IC):
                    xt9 = xt9p.tile([K9, HW], bf16, tag="xt9")
                    nc.sync.dma_start(out=xt9, in_=xt9_d[b, i])
                    xv = xt9.rearrange("p (h w) -> p h w", w=W)
                    U_i = utmpp.tile([128, 2, HP], bf16, tag="ut")
                    e_i = etmpp.tile([128, 2, HP], bf16, tag="et")
                    pz = pzp.tile([128, 2, 512], f32, tag="pz")
                    for h in range(2):
                        pu = pup.tile([128, 512], f32, tag="pu")
                        nc.tensor.matmul(
                            pu[:, 0:HP],
                            w72s,
                            xv[:, 15 * h : 15 * h + 15, 0:OW],
                            start=True,
                            stop=True,
                        )
                        if (2 * i + h) % 2 == 0:
                            nc.vector.tensor_copy(out=U_i[:, h, :], in_=pu[:, 0:HP])
                        else:
                            nc.scalar.copy(out=U_i[:, h, :], in_=pu[:, 0:HP])
                    q = qp.tile([128, 2, HP], bf16, tag="q")
                    nc.vector.tensor_mul(out=q, in0=U_i, in1=t_bf)
                    for h in range(2):
                        nc.tensor.matmul(
                            pz[:, h, 0:HP], lreps, q[:, h, :], start=True, stop=True
                        )
                    nc.scalar.activation(
                        out=e_i,
                        in_=pz[:, :, 0:HP],
                        func=AF.Exp,
                        scale=RSQRT_L,
                        accum_out=colsum[:, i : i + 1],
                    )
                    q2_i = q2p.tile([128, 2, HP], bf16, tag="q2", name=f"q2_{i}")
                    q2_tiles.append(q2_i)
                    nc.vector.tensor_mul(out=q2_i, in0=U_i, in1=e_i)

                # ---------- softmax denominator: allreduce over partitions ----------
                from concourse import bass_isa

                s_all = smp.tile([128, IC], f32, tag="sall")
                nc.gpsimd.partition_all_reduce(
                    s_all, colsum, 128, bass_isa.ReduceOp.add
                )
                sinv_tab = smp.tile([128, IC], f32, tag="stab")
                nc.vector.reciprocal(out=sinv_tab, in_=s_all)

                # ---------- s phase + squash, one half at a time ----------
                si_tiles = []
                si_eng = nc.gpsimd if b == 0 else nc.vector
                for i in range(IC):
                    si = sip.tile([128, 128], bf16, tag="si", name=f"si{i}")
                    si_eng.tensor_scalar(
                        out=si,
                        in0=i128s,
                        scalar1=sinv_tab[:, i : i + 1],
                        scalar2=float(L),
                        op0=OP.mult,
                        op1=OP.mult,
                    )
                    si_tiles.append(si)
                o_t = sqp.tile([128, 2, HP], f32, tag="ot")
                for h in range(2):
                    psum_s = ltp.tile([128, 512], f32, tag="lt")
                    for i in range(IC):
                        nc.tensor.matmul(
                            psum_s[:, 0:HP],
                            si_tiles[i],
                            q2_tiles[i][:, h, :],
                            start=(i == 0),
                            stop=(i == IC - 1),
                        )
                    v_sb = sqp.tile([128, HP], f32, tag="vsb")
                    nc.vector.tensor_add(
                        out=v_sb, in0=psum_s[:, 0:HP], in1=t2_f[:, h, :]
                    )
                    sq_bf = sqp.tile([128, HP], bf16, tag="sqbf")
                    nc.scalar.activation(out=sq_bf, in_=v_sb, func=AF.Square)
                    pn = ltp.tile([128, 512], f32, tag="lt")
                    nc.tensor.matmul(
                        pn[:, 0:HP], lreps, sq_bf, start=True, stop=True
                    )
                    rsb = sqp.tile([128, HP], f32, tag="rsb")
                    nc.scalar.activation(
                        out=rsb, in_=pn[:, 0:HP], func=AF.Sqrt, bias=eps_t
                    )
                    rinv = sqp.tile([128, HP], f32, tag="rinv")
                    nc.vector.reciprocal(out=rinv, in_=rsb)
                    g_t = sqp.tile([128, HP], f32, tag="gt")
                    nc.scalar.activation(out=g_t, in_=rsb, func=AF.Exp, scale=-1.0)
                    nc.vector.tensor_scalar(
                        out=g_t,
                        in0=g_t,
                        scalar1=-1.0,
                        scalar2=1.0,
                        op0=OP.mult,
                        op1=OP.add,
                    )
                    a_t = sqp.tile([128, HP], f32, tag="at")
                    nc.vector.tensor_mul(out=a_t, in0=v_sb, in1=rinv)
                    nc.vector.tensor_mul(out=o_t[:, h, :], in0=a_t, in1=g_t)
                nc.gpsimd.dma_start(
                    out=y_d[b].rearrange("p (h n) -> p h n", h=2), in_=o_t
                )

    nc.finalize()
    return nc


def _prep_host(x, w, b_route):
    import ml_dtypes

    bf = ml_dtypes.bfloat16
    x = np.ascontiguousarray(np.asarray(x, dtype=np.float32))
    w = np.asarray(w, dtype=np.float32)
    b_route = np.asarray(b_route, dtype=np.float32)

    # xt[b, i, il, hw]
    xt = np.ascontiguousarray(x.transpose(0, 3, 4, 1, 2)).reshape(B, IC, IL, HW)
    xt9 = np.zeros((B, IC, K9, HW), dtype=bf)
    xtb = xt.astype(bf)
    for g, s in enumerate(SHIFTS):
        if s == 0:
            xt9[:, :, g * IL : (g + 1) * IL, :] = xtb
        else:
            xt9[:, :, g * IL : (g + 1) * IL, : HW - s] = xtb[:, :, :, s:]

    xnat = x.reshape(B, HW, IC * IL)

    # W72[(ky,kx,il), cl]
    w72 = np.ascontiguousarray(
        w[:, :, :, 0, :].transpose(1, 2, 0, 3).reshape(K9, CL)
    ).astype(bf)
    lrep = np.kron(np.eye(C, dtype=np.float32), np.ones((L, L), np.float32)).astype(bf)
    i128 = np.eye(128, dtype=np.float32).astype(bf)
    # br_cl[(c*8+l), pos] = b_route[pos*16+c, l]
    br_cl = np.ascontiguousarray(
        b_route.reshape(POS, C, L).transpose(1, 2, 0).reshape(128, POS)
    ).astype(np.float32)
    return xt9, xnat, w72, lrep, i128, br_cl


def kernel(x, w, b_route, stride):
    assert int(stride) == 1
    xt9, xnat, w72, lrep, i128, br_cl = _prep_host(x, w, b_route)

    if "nc" not in _CACHE:
        _CACHE["nc"] = _build_nc()
    nc = _CACHE["nc"]

    from concourse.bass_utils import run_bass_kernel_spmd

    in_maps = []
    for c in range(NCORES):
        sl = slice(c * BLOC, (c + 1) * BLOC)
        in_maps.append(
            {
                "xt9": np.ascontiguousarray(xt9[sl]),
                "xnat": np.ascontiguousarray(xnat[sl]),
                "w72": w72,
                "lrep": lrep,
                "i128": i128,
                "br_cl": br_cl,
            }
        )

    res = run_bass_kernel_spmd(nc, in_maps, core_ids=list(range(NCORES)))

    y = np.empty((B, OH, OW, C, L), dtype=np.float32)
    for c in range(NCORES):
        yd = res.results[c]["y"]  # [BLOC, 128, 900]
        y[c * BLOC : (c + 1) * BLOC] = (
            yd.reshape(BLOC, C, L, POS).transpose(0, 3, 1, 2).reshape(
                BLOC, OH, OW, C, L
            )
        )
    return y


# revision 45
# speedup vs baseline: 1.0225x; 1.0062x over previous
"""Trainium2 Bass/Tile kernel for nn_Capsule3D (capsule conv + routing softmax + squash).

Sharding: data-parallel over batch, 2 samples per core x 8 cores. Host side does
only layout transforms (transpose / 9-shift im2col row replication / dtype casts)
and sharding; all math runs on the NeuronCores.

Per sample b, on device (layout: partitions = (c,l) = 128 output channels,
free = output positions pos = 900, per input capsule i = 0..31):
  - t = conv(sum_i x_i) via a mini 72x128 matmul (conv is linear in its input,
    so the routing sum over capsules commutes with the conv)
  - main loop per i: K=72 weights-stationary conv matmul -> PSUM; evict to bf16
    (split ScalarE/VectorE); q = u_hat*t (VectorE bf16 2x); "Lrep" matmul with a
    block-diagonal ones matrix reduces over l AND replicates the result over the
    l partitions; exp((z)/sqrt L) on ScalarE straight from PSUM with accum_out
    giving the softmax denominator column-sums for free; q2 = u_hat*e (VectorE).
  - softmax denominators via gpsimd partition_all_reduce; 1/S_i folded into
    per-i scaled-identity matmuls ("si") built on GpSimd.
  - s = sum_i si^T @ q2_i accumulated in PSUM by TensorE (+ t*b_route term),
    so the s-phase is matmul-only and overlaps the next sample's main loop.
  - squash: norm over l via Lrep matmul on v^2, then v*(1-exp(-r))/r.

The softmax skips the max-subtraction (logits are O(5), safe in fp32 exp).
Intermediates are bf16 (measured end-to-end error ~9e-3 scale-relative absmax
vs the fp32 reference, i.e. ~1e-2 l2-relative, under the 2e-2 gate).
"""

import math

import numpy as np

# ---------------- problem constants (hardcoded per harness contract) ----------
B, H, W, IC, IL = 16, 32, 32, 32, 8
KH = KW = 3
CL = 128
L = 8
C = CL // L            # 16
OH = OW = 30
POS = OH * OW          # 900
HW = H * W             # 1024
K9 = KH * KW * IL      # 72
NCORES = 8
BLOC = B // NCORES     # 2
EPS = 1e-7
RSQRT_L = 1.0 / math.sqrt(float(L))
SHIFTS = [32 * ky + kx for ky in range(KH) for kx in range(KW)]

_CACHE = {}


def _build_nc():
    import concourse.tile as tile
    from concourse import bacc, mybir

    f32 = mybir.dt.float32
    bf16 = mybir.dt.bfloat16
    AF = mybir.ActivationFunctionType
    OP = mybir.AluOpType

    nc = bacc.Bacc()

    xt9_d = nc.dram_tensor("xt9", [BLOC, IC, K9, HW], bf16, kind="ExternalInput")
    xnat_d = nc.dram_tensor("xnat", [BLOC, HW, IC * IL], f32, kind="ExternalInput")
    w72_d = nc.dram_tensor("w72", [K9, CL], bf16, kind="ExternalInput")
    lrep_d = nc.dram_tensor("lrep", [128, 128], bf16, kind="ExternalInput")
    i128_d = nc.dram_tensor("i128", [128, 128], bf16, kind="ExternalInput")
    br_d = nc.dram_tensor("br_cl", [128, POS], f32, kind="ExternalInput")
    y_d = nc.dram_tensor("y", [BLOC, 128, POS], f32, kind="ExternalOutput")

    HP = 450  # half of the 900 output positions

    with tile.TileContext(nc) as tc:
        with (
            tc.tile_pool(name="const", bufs=1) as constp,
            tc.tile_pool(name="xnat", bufs=2) as xnatp,
            tc.tile_pool(name="ub", bufs=4) as ubp,
            tc.tile_pool(name="ubar", bufs=1) as ubarp,
            tc.tile_pool(name="xt9", bufs=4) as xt9p,
            tc.tile_pool(name="utmp", bufs=5) as utmpp,
            tc.tile_pool(name="etmp", bufs=5) as etmpp,
            tc.tile_pool(name="q2s", bufs=IC) as q2p,
            tc.tile_pool(name="tt", bufs=2) as ttp,
            tc.tile_pool(name="q", bufs=8) as qp,
            tc.tile_pool(name="sip", bufs=IC) as sip,
            tc.tile_pool(name="sm", bufs=2) as smp,
            tc.tile_pool(name="sq", bufs=3) as sqp,
            tc.tile_pool(name="pu", bufs=3, space="PSUM") as pup,
            tc.tile_pool(name="pz", bufs=2, space="PSUM") as pzp,
            tc.tile_pool(name="lt", bufs=1, space="PSUM") as ltp,
        ):
            # ---- constants (loaded once) ----
            w72s = constp.tile([K9, CL], bf16)
            nc.sync.dma_start(out=w72s, in_=w72_d[:, :])
            lreps = constp.tile([128, 128], bf16)
            nc.sync.dma_start(out=lreps, in_=lrep_d[:, :])
            i128s = constp.tile([128, 128], bf16)
            nc.sync.dma_start(out=i128s, in_=i128_d[:, :])
            brs = constp.tile([128, POS], f32)
            nc.gpsimd.dma_start(out=brs, in_=br_d[:, :])
            eps_t = constp.tile([128, 1], f32)
            nc.vector.memset(eps_t, EPS)
            eye_f = constp.tile([128, 128], f32)
            nc.vector.tensor_copy(out=eye_f, in_=i128s)

            for b in range(BLOC):
                # ---------- ubar path ----------
                ubarT = ubarp.tile([IL, HW], bf16, tag="ubarT")
                xn = xnatp.tile([128, HW // 128, IC * IL], f32, tag="xn")
                xnv = xnat_d[b].rearrange("(t p) f -> p t f", p=128)
                nc.sync.dma_start(out=xn[:, 0:4, :], in_=xnv[:, 0:4, :])
                nc.sync.dma_start(out=xn[:, 4:8, :], in_=xnv[:, 4:8, :])
                for hwt in range(HW // 128):
                    ub_f = ubp.tile([128, IL], f32, tag="ubf")
                    nc.vector.reduce_sum(
                        out=ub_f,
                        in_=xn[:, hwt, :].rearrange("p (i l) -> p l i", l=IL),
                        axis=mybir.AxisListType.X,
                    )
                    ps_tr = pup.tile([IL, 128], f32, tag="pu")
                    nc.tensor.transpose(ps_tr, ub_f, eye_f)
                    if hwt % 2 == 0:
                        nc.vector.tensor_copy(
                            out=ubarT[:, hwt * 128 : (hwt + 1) * 128], in_=ps_tr
                        )
                    else:
                        nc.scalar.copy(
                            out=ubarT[:, hwt * 128 : (hwt + 1) * 128], in_=ps_tr
                        )
                ubar9 = ubarp.tile([K9, HW], bf16, tag="ubar9")
                for g, s in enumerate(SHIFTS):
                    nc.sync.dma_start(
                        out=ubar9[g * IL : (g + 1) * IL, 0 : HW - s],
                        in_=ubarT[:, s:HW],
                    )

                # ---------- t mini-conv (per half) ----------
                ub_v = ubar9.rearrange("p (h w) -> p h w", w=W)
                t_bf = ttp.tile([128, 2, HP], bf16, tag="tbf")
                t2_f = ttp.tile([128, 2, HP], f32, tag="t2")
                brv = brs.rearrange("p (h n) -> p h n", h=2)
                psum_t = pzp.tile([128, 2, 512], f32, tag="pz")
                for h in range(2):
                    nc.tensor.matmul(
                        psum_t[:, h, 0:HP],
                        w72s,
                        ub_v[:, 15 * h : 15 * h + 15, 0:OW],
                        start=True,
                        stop=True,
                    )
                    nc.scalar.copy(out=t_bf[:, h, :], in_=psum_t[:, h, 0:HP])
                    nc.vector.tensor_mul(
                        out=t2_f[:, h, :], in0=psum_t[:, h, 0:HP], in1=brv[:, h, :]
                    )

                # ---------- main loop: conv, evict, z, l-reduce, exp, q2 ----------
                q2_tiles = []
                colsum = smp.tile([128, IC], f32, tag="colsum")
                for i in range(IC):
                    xt9 = xt9p.tile([K9, HW], bf16, tag="xt9")
                    nc.sync.dma_start(out=xt9, in_=xt9_d[b, i])
                    xv = xt9.rearrange("p (h w) -> p h w", w=W)
                    U_i = utmpp.tile([128, 2, HP], bf16, tag="ut")
                    e_i = etmpp.tile([128, 2, HP], bf16, tag="et")
                    pz = pzp.tile([128, 2, 512], f32, tag="pz")
                    for h in range(2):
                        pu = pup.tile([128, 512], f32, tag="pu")
                        nc.tensor.matmul(
                            pu[:, 0:HP],
                            w72s,
                            xv[:, 15 * h : 15 * h + 15, 0:OW],
                            start=True,
                            stop=True,
                        )
                        if (2 * i + h) % 2 == 0:
                            nc.vector.tensor_copy(out=U_i[:, h, :], in_=pu[:, 0:HP])
                        else:
                            nc.scalar.copy(out=U_i[:, h, :], in_=pu[:, 0:HP])
                    q = qp.tile([128, 2, HP], bf16, tag="q")
                    nc.vector.tensor_mul(out=q, in0=U_i, in1=t_bf)
                    for h in range(2):
                        nc.tensor.matmul(
                            pz[:, h, 0:HP], lreps, q[:, h, :], start=True, stop=True
                        )
                    nc.scalar.activation(
                        out=e_i,
                        in_=pz[:, :, 0:HP],
                        func=AF.Exp,
                        scale=RSQRT_L,
                        accum_out=colsum[:, i : i + 1],
                    )
                    q2_i = q2p.tile([128, 2, HP], bf16, tag="q2", name=f"q2_{i}")
                    q2_tiles.append(q2_i)
                    nc.vector.tensor_mul(out=q2_i, in0=U_i, in1=e_i)

                # ---------- softmax denominator: allreduce over partitions ----------
                from concourse import bass_isa

                s_all = smp.tile([128, IC], f32, tag="sall")
                nc.gpsimd.partition_all_reduce(
                    s_all, colsum, 128, bass_isa.ReduceOp.add
                )
                sinv_tab = smp.tile([128, IC], f32, tag="stab")
                nc.vector.reciprocal(out=sinv_tab, in_=s_all)

                # ---------- s phase + squash, one half at a time ----------
                si_tiles = []
                for i in range(IC):
                    si = sip.tile([128, 128], bf16, tag="si", name=f"si{i}")
                    si_eng = nc.gpsimd if (i + b) % 2 == 0 else nc.vector
                    si_eng.tensor_scalar(
                        out=si,
                        in0=i128s,
                        scalar1=sinv_tab[:, i : i + 1],
                        scalar2=float(L),
                        op0=OP.mult,
                        op1=OP.mult,
                    )
                    si_tiles.append(si)
                o_t = sqp.tile([128, 2, HP], f32, tag="ot")
                for h in range(2):
                    psum_s = ltp.tile([128, 512], f32, tag="lt")
                    for i in range(IC):
                        nc.tensor.matmul(
                            psum_s[:, 0:HP],
                            si_tiles[i],
                            q2_tiles[i][:, h, :],
                            start=(i == 0),
                            stop=(i == IC - 1),
                        )
                    v_sb = sqp.tile([128, HP], f32, tag="vsb")
                    nc.vector.tensor_add(
                        out=v_sb, in0=psum_s[:, 0:HP], in1=t2_f[:, h, :]
                    )
                    sq_bf = sqp.tile([128, HP], bf16, tag="sqbf")
                    nc.scalar.activation(out=sq_bf, in_=v_sb, func=AF.Square)
                    pn = ltp.tile([128, 512], f32, tag="lt")
                    nc.tensor.matmul(
                        pn[:, 0:HP], lreps, sq_bf, start=True, stop=True
                    )
                    rsb = sqp.tile([128, HP], f32, tag="rsb")
                    nc.scalar.activation(
                        out=rsb, in_=pn[:, 0:HP], func=AF.Sqrt, bias=eps_t
                    )
                    rinv = sqp.tile([128, HP], f32, tag="rinv")
                    nc.vector.reciprocal(out=rinv, in_=rsb)
                    g_t = sqp.tile([128, HP], f32, tag="gt")
                    nc.scalar.activation(out=g_t, in_=rsb, func=AF.Exp, scale=-1.0)
                    nc.vector.tensor_scalar(
                        out=g_t,
                        in0=g_t,
                        scalar1=-1.0,
                        scalar2=1.0,
                        op0=OP.mult,
                        op1=OP.add,
                    )
                    a_t = sqp.tile([128, HP], f32, tag="at")
                    nc.vector.tensor_mul(out=a_t, in0=v_sb, in1=rinv)
                    nc.vector.tensor_mul(out=o_t[:, h, :], in0=a_t, in1=g_t)
                nc.gpsimd.dma_start(
                    out=y_d[b].rearrange("p (h n) -> p h n", h=2), in_=o_t
                )

    nc.finalize()
    return nc


def _prep_host(x, w, b_route):
    import ml_dtypes

    bf = ml_dtypes.bfloat16
    x = np.ascontiguousarray(np.asarray(x, dtype=np.float32))
    w = np.asarray(w, dtype=np.float32)
    b_route = np.asarray(b_route, dtype=np.float32)

    # xt[b, i, il, hw]
    xt = np.ascontiguousarray(x.transpose(0, 3, 4, 1, 2)).reshape(B, IC, IL, HW)
    xt9 = np.zeros((B, IC, K9, HW), dtype=bf)
    xtb = xt.astype(bf)
    for g, s in enumerate(SHIFTS):
        if s == 0:
            xt9[:, :, g * IL : (g + 1) * IL, :] = xtb
        else:
            xt9[:, :, g * IL : (g + 1) * IL, : HW - s] = xtb[:, :, :, s:]

    xnat = x.reshape(B, HW, IC * IL)

    # W72[(ky,kx,il), cl]
    w72 = np.ascontiguousarray(
        w[:, :, :, 0, :].transpose(1, 2, 0, 3).reshape(K9, CL)
    ).astype(bf)
    lrep = np.kron(np.eye(C, dtype=np.float32), np.ones((L, L), np.float32)).astype(bf)
    i128 = np.eye(128, dtype=np.float32).astype(bf)
    # br_cl[(c*8+l), pos] = b_route[pos*16+c, l]
    br_cl = np.ascontiguousarray(
        b_route.reshape(POS, C, L).transpose(1, 2, 0).reshape(128, POS)
    ).astype(np.float32)
    return xt9, xnat, w72, lrep, i128, br_cl


def kernel(x, w, b_route, stride):
    assert int(stride) == 1
    xt9, xnat, w72, lrep, i128, br_cl = _prep_host(x, w, b_route)

    if "nc" not in _CACHE:
        _CACHE["nc"] = _build_nc()
    nc = _CACHE["nc"]

    from concourse.bass_utils import run_bass_kernel_spmd

    in_maps = []
    for c in range(NCORES):
        sl = slice(c * BLOC, (c + 1) * BLOC)
        in_maps.append(
            {
                "xt9": np.ascontiguousarray(xt9[sl]),
                "xnat": np.ascontiguousarray(xnat[sl]),
                "w72": w72,
                "lrep": lrep,
                "i128": i128,
                "br_cl": br_cl,
            }
        )

    res = run_bass_kernel_spmd(nc, in_maps, core_ids=list(range(NCORES)))

    y = np.empty((B, OH, OW, C, L), dtype=np.float32)
    for c in range(NCORES):
        yd = res.results[c]["y"]  # [BLOC, 128, 900]
        y[c * BLOC : (c + 1) * BLOC] = (
            yd.reshape(BLOC, C, L, POS).transpose(0, 3, 1, 2).reshape(
                BLOC, OH, OW, C, L
            )
        )
    return y
